# revision 11
# baseline (speedup 1.0000x reference)
"""LocallyConnected1d (B=32, C=32, L=4096, K=7, stride=1) Trainium2 Bass kernel.

Strategy (hardcoded for this problem):
  - Shard L_out=4090 across 8 cores (sequence parallel), 512 positions/core
    (padded; core 7 carries 6 zero-padded positions). Weights are read from
    HBM exactly once, in bf16 (rel-err budget 2e-2 >> bf16 rounding).
  - Host pre-permutes operands into PE-friendly bf16 layouts:
      x2 [128, 516*32]: partition (tap-band kk 0..3, in_C i), col (c, b),
                        value x[b, i, 512m + c + kk]
      wf [128, 516*64]: partition (kk, i), col (c, h, o):
                        h=0 ("B" half) w[o, i, 512m+c-4, 4+kk] (kk=3 -> 0)
                        h=1 ("A" half) w[o, i, 512m+c, kk]
  - One stationary column c serves TWO positions: taps 0..3 of position c
    (A) and taps 4..6 of position c-4 (B, with a zero 4th tap band). So one
    LDWEIGHTS (x2 col c) + one N=64 matmul per column replaces the two
    N=32 matmuls of the naive im2col split:
      psum[b, {B: pos c-4 | A: pos c}] += x2[:, c]^T . wf[:, c]
  - PSUM: position p -> col group cg=p%4 (psum partitions 32cg..32cg+31
    hold b), bank g=p//32, pair-slot j=(p//4)%8: A half at cols
    [64j,64j+32), B half at [64j+32,64j+64). Every psum element is written
    by exactly ONE matmul (start=stop=True), so there are no accumulation
    groups at all; the drain adds A+B on VectorE while casting to bf16.
    Banks ping-pong (bufs=2); output leaves in 4 chunked bf16 DMAs (host
    adds nothing further, just un-permutes and upcasts to f32).
"""

import sys

if "/opt/trn_rl_repo" not in sys.path:
    sys.path.insert(0, "/opt/trn_rl_repo")

import numpy as np

import bass_rust
from concourse import bass, mybir, tile
from concourse.bass_utils import run_bass_kernel_spmd

# Problem constants (hardcoded; must match the grading reference).
B = 32          # batch
IC = 32         # in channels
L = 4096        # input length
OC = 32         # out channels
K = 7           # kernel taps
L_OUT = 4090    # (L - (K-1)) // 1

NCORES = 8
LP = 512        # positions per core (padded: 8*512 = 4096 >= 4090)
NC_ = LP + 4    # stationary columns per core (c = 0..515)
# weight chunk column boundaries: big steady-state chunks, small final chunk
# so the tail (matmuls that can only start after the last chunk lands) is short
WCHUNKS = [0, 120, 240, 340, 420, 470, 500, 516]
WCMAX = 120     # max chunk width (tile allocation size)
# output DMA bank-group boundaries (last group small -> small tail DMA)
OGROUPS = [0, 5, 10, 14, 16]

X2COLS = NC_ * B        # x2 cols: c*32 + b
WCOLS = NC_ * 64        # wf cols: c*64 + h*32 + o
OCOLS = 16 * 256        # ostage cols: g*256 + j*32 + o   (= 4096)

F32 = mybir.dt.float32
BF16 = mybir.dt.bfloat16

_CACHE = {}


def _ap(t_ap, offset, dims):
    """Build a raw access pattern on the tensor behind an AP."""
    return bass_rust.AP(t_ap.tensor, int(offset), [[int(s), int(n)] for s, n in dims])


def _emit(reps=None):
    """Build the (identical-per-core) single-core program.

    reps: if set, wrap the whole body (DMAs included) in a hardware loop that
    executes it `reps` times -- used only for wall-clock timing calibration.
    """
    import contextlib

    nc = bass.Bass()
    x_d = nc.dram_tensor("x2", [128, X2COLS], BF16, kind="ExternalInput")
    w_d = nc.dram_tensor("wf", [128, WCOLS], BF16, kind="ExternalInput")
    o_d = nc.dram_tensor("out", [128, OCOLS], BF16, kind="ExternalOutput")

    with tile.TileContext(nc) as tc:
        with (
            tc.tile_pool(name="persist", bufs=1) as persist,
            tc.tile_pool(name="wpool", bufs=3) as wpool,
            tc.tile_pool(name="btmp", bufs=2) as btmp,
            tc.tile_pool(name="psum", bufs=2, space=bass.MemorySpace.PSUM) as psum,
        ):
            x2 = persist.tile([128, X2COLS], BF16, name="x2t")
            x2a = x2[:]
            ost = persist.tile([128, OCOLS], BF16, name="ostage")
            osa = ost[:]

            loop = (
                tc.For_i(0, reps, 1, hint_engines=(mybir.EngineType.PE,))
                if reps is not None else contextlib.nullcontext()
            )
            with loop:
                _emit_body(nc, x2a, osa, x_d, w_d, o_d, wpool, btmp, psum)
    _split_matmul_waits(nc)
    return nc


def _emit_body(nc, x2a, osa, x_d, w_d, o_d, wpool, btmp, psum):
    nc.sync.dma_start(x2a, x_d[:])

    nch = len(WCHUNKS) - 1
    wc = [None] * nch
    banks = [None] * 16  # psum bank APs (pool cycles 2 physical buffers)
    c2ch = np.searchsorted(WCHUNKS, np.arange(NC_), side="right") - 1

    for c in range(NC_):
        jch = int(c2ch[c])
        c_loc = c - WCHUNKS[jch]
        q, cg = divmod(c, 4)
        g, j = divmod(q, 8)   # bank / pair-slot of position c's A half

        if c_loc == 0:
            ncols = (WCHUNKS[jch + 1] - WCHUNKS[jch]) * 64
            wt = wpool.tile([128, WCMAX * 64], BF16, tag="wc", name=f"wc{jch}")
            wc[jch] = wt[:]
            nc.sync.dma_start(
                _ap(wc[jch], 0, [[WCMAX * 64, 128], [1, ncols]]),
                _ap(w_d[:], WCHUNKS[jch] * 64, [[WCOLS, 128], [1, ncols]]),
            )
        if c % 32 == 0 and c < LP:
            pgt = psum.tile([128, 512], F32, tag="ps", name=f"ps{g}")
            banks[g] = pgt[:]

        lhs = _ap(x2a, c * B, [[X2COLS, 128], [1, B]])
        wofs = c_loc * 64
        fused = 4 <= c < LP and j != 0
        if fused:
            # one N=64 matmul: [B half of pos c-4 | A half of pos c]
            nc.tensor.matmul(
                _ap(banks[g], 32 * cg * 512 + 64 * j - 32, [[512, 32], [1, 64]]),
                lhs,
                _ap(wc[jch], wofs, [[WCMAX * 64, 128], [1, 64]]),
                start=True, stop=True,
                tile_position=(0, 32 * cg), skip_group_check=True,
            )
        else:
            if c >= 4:
                # B half of pos c-4 -> pair-slot 7 of the previous bank
                nc.tensor.matmul(
                    _ap(banks[(c - 4) // 32], 32 * cg * 512 + 480,
                        [[512, 32], [1, 32]]),
                    lhs,
                    _ap(wc[jch], wofs, [[WCMAX * 64, 128], [1, 32]]),
                    start=True, stop=True,
                    tile_position=(0, 32 * cg), skip_group_check=True,
                )
            if c < LP:
                # A half of pos c -> pair-slot 0 of the current bank
                nc.tensor.matmul(
                    _ap(banks[g], 32 * cg * 512 + 64 * j, [[512, 32], [1, 32]]),
                    lhs,
                    _ap(wc[jch], wofs + 32, [[WCMAX * 64, 128], [1, 32]]),
                    start=True, stop=True,
                    tile_position=(0, 32 * cg), skip_group_check=True,
                )

        if c >= 35 and c % 32 == 3:
            gd = (c - 35) // 32  # bank whose last write (B of pos 32gd+31) was c
            # TensorTensor may read only ONE input from PSUM: stage the B
            # halves into SBUF first (bf16), then add the A halves (PSUM).
            bt = btmp.tile([128, 256], BF16, tag="bt", name=f"bt{gd}")
            bta = bt[:]
            nc.vector.tensor_copy(
                _ap(bta, 0, [[256, 128], [32, 8], [1, 32]]),
                _ap(banks[gd], 32, [[512, 128], [64, 8], [1, 32]]),
            )
            nc.vector.tensor_tensor(
                _ap(osa, gd * 256, [[OCOLS, 128], [32, 8], [1, 32]]),
                _ap(banks[gd], 0, [[512, 128], [64, 8], [1, 32]]),
                _ap(bta, 0, [[256, 128], [32, 8], [1, 32]]),
                mybir.AluOpType.add,
            )
            if gd + 1 in OGROUPS:
                k = OGROUPS.index(gd + 1) - 1
                lo, hi = OGROUPS[k] * 256, OGROUPS[k + 1] * 256
                nc.sync.dma_start(
                    _ap(o_d[:], lo, [[OCOLS, 128], [1, hi - lo]]),
                    _ap(osa, lo, [[OCOLS, 128], [1, hi - lo]]),
                )


def _split_matmul_waits(nc):
    """This walrus build allows at most one sync wait per instruction.
    Relocate each multi-wait instruction's waits onto a chain of single-wait
    NoOps inserted just before it on the same engine -- program order makes
    this semantically identical."""
    for f in nc.m.functions:
        for bb in f.blocks:
            insts = list(bb.instructions)
            out = []
            changed = False
            for ins in insts:
                si = ins.sync_info
                if (si is not None and si.on_wait
                        and len(si.on_wait) >= 2):
                    for w in si.on_wait:
                        nop = mybir.InstNoOp(
                            name=nc.get_next_instruction_name(),
                            ins=[], outs=[],
                            sync_info=mybir.SyncInfo(
                                on_wait=[w], on_update=[]),
                            bass_nofuse=True,
                            engine=ins.engine,
                        )
                        nc.inst_map[nop.name] = nop
                        out.append(nop)
                    ins.sync_info = mybir.SyncInfo(
                        on_wait=[], on_update=list(si.on_update))
                    changed = True
                out.append(ins)
            if changed:
                bb.instructions = out


def _get_nc():
    if "nc" not in _CACHE:
        _CACHE["nc"] = _emit()
    return _CACHE["nc"]


def _shard_inputs(x, weight):
    """Pre-permute full inputs into the per-core bf16 kernel layouts."""
    import ml_dtypes

    bf16 = ml_dtypes.bfloat16
    x = np.asarray(x, dtype=np.float32)
    weight = np.asarray(weight, dtype=np.float32)

    xpad = np.zeros((B, IC, NCORES * LP + NC_ + 4), dtype=np.float32)
    xpad[:, :, :L] = x
    # wglob: 4-zero L prefix so the B-half window (l0+c-4) never underflows,
    # an 8th zero tap, and an L suffix past L_OUT.
    wglob = np.zeros((OC, IC, 4 + NCORES * LP + NC_, 8), dtype=np.float32)
    wglob[:, :, 4 : 4 + L_OUT, :K] = weight

    in_maps = []
    for m in range(NCORES):
        l0 = m * LP
        x2 = np.empty((4, IC, NC_, B), dtype=bf16)
        for kk in range(4):
            x2[kk] = xpad[:, :, l0 + kk : l0 + kk + NC_].transpose(1, 2, 0)
        wA = wglob[:, :, 4 + l0 : 4 + l0 + NC_, 0:4]  # (O,I,c,kk)
        wB = wglob[:, :, l0 : l0 + NC_, 4:8]
        wf = np.empty((4, IC, NC_, 2, OC), dtype=bf16)
        wf[:, :, :, 0, :] = wB.transpose(3, 1, 2, 0)
        wf[:, :, :, 1, :] = wA.transpose(3, 1, 2, 0)
        in_maps.append({
            "x2": np.ascontiguousarray(x2).reshape(128, X2COLS),
            "wf": np.ascontiguousarray(wf).reshape(128, WCOLS),
        })
    return in_maps


def _unshard_output(res):
    """res: list of per-core {"out": (128, OCOLS) bf16} -> (B, OC, L_OUT) f32."""
    out = np.empty((B, OC, NCORES * LP), dtype=np.float32)
    for m in range(NCORES):
        arr = np.asarray(res[m]["out"], dtype=np.float32)
        arr = arr.reshape(4, B, 16, 8, OC)          # (cg, b, g, j, o)
        out[:, :, m * LP : (m + 1) * LP] = (
            arr.transpose(1, 4, 2, 3, 0).reshape(B, OC, LP)
        )
    return np.ascontiguousarray(out[:, :, :L_OUT])


def kernel(x, weight):
    nc = _get_nc()
    in_maps = _shard_inputs(x, weight)
    res = run_bass_kernel_spmd(nc, in_maps, list(range(NCORES))).results
    return _unshard_output(res)


# revision 12
# speedup vs baseline: 2.3160x; 2.3160x over previous
"""LocallyConnected1d (B=32, C=32, L=4096, K=7, stride=1) Trainium2 Bass kernel.

Strategy (hardcoded for this problem):
  - Shard L_out=4090 across 8 cores (sequence parallel), 512 positions/core
    (padded; core 7 carries 6 zero-padded positions). Weights are read from
    HBM exactly once, in bf16 (rel-err budget 2e-2 >> bf16 rounding).
  - Host pre-permutes operands into PE-friendly bf16 layouts:
      x2 [128, 516*32]: partition (tap-band kk 0..3, in_C i), col (c, b),
                        value x[b, i, 512m + c + kk]
      wf [128, 516*64]: partition (kk, i), col (c, h, o):
                        h=0 ("B" half) w[o, i, 512m+c-4, 4+kk] (kk=3 -> 0)
                        h=1 ("A" half) w[o, i, 512m+c, kk]
  - One stationary column c serves TWO positions: taps 0..3 of position c
    (A) and taps 4..6 of position c-4 (B, with a zero 4th tap band). So one
    LDWEIGHTS (x2 col c) + one N=64 matmul per column replaces the two
    N=32 matmuls of the naive im2col split:
      psum[b, {B: pos c-4 | A: pos c}] += x2[:, c]^T . wf[:, c]
  - PSUM: position p -> col group cg=p%4 (psum partitions 32cg..32cg+31
    hold b), bank g=p//32, pair-slot j=(p//4)%8: A half at cols
    [64j,64j+32), B half at [64j+32,64j+64). Every psum element is written
    by exactly ONE matmul (start=stop=True), so there are no accumulation
    groups at all; the drain adds A+B on VectorE while casting to bf16.
    Banks ping-pong (bufs=2); output leaves in 4 chunked bf16 DMAs (host
    adds nothing further, just un-permutes and upcasts to f32).
"""

import sys

if "/opt/trn_rl_repo" not in sys.path:
    sys.path.insert(0, "/opt/trn_rl_repo")

import numpy as np

import bass_rust
from concourse import bass, mybir, tile
from concourse.bass_utils import run_bass_kernel_spmd

# Problem constants (hardcoded; must match the grading reference).
B = 32          # batch
IC = 32         # in channels
L = 4096        # input length
OC = 32         # out channels
K = 7           # kernel taps
L_OUT = 4090    # (L - (K-1)) // 1

NCORES = 8
LP = 512        # positions per core (padded: 8*512 = 4096 >= 4090)
NC_ = LP + 4    # stationary columns per core (c = 0..515)
# weight chunk column boundaries: big steady-state chunks, small final chunk
# so the tail (matmuls that can only start after the last chunk lands) is short
WCHUNKS = [0, 120, 240, 340, 420, 470, 500, 516]
WCMAX = 120     # max chunk width (tile allocation size)
# output DMA bank-group boundaries (last group small -> small tail DMA)
OGROUPS = [0, 5, 10, 14, 16]

X2COLS = NC_ * B        # x2 cols: c*32 + b
WCOLS = NC_ * 64        # wf cols: c*64 + h*32 + o
OCOLS = 16 * 256        # ostage cols: g*256 + j*32 + o   (= 4096)

F32 = mybir.dt.float32
BF16 = mybir.dt.bfloat16

_CACHE = {}


def _ap(t_ap, offset, dims):
    """Build a raw access pattern on the tensor behind an AP."""
    return bass_rust.AP(t_ap.tensor, int(offset), [[int(s), int(n)] for s, n in dims])


def _emit(reps=None):
    """Build the (identical-per-core) single-core program.

    reps: if set, wrap the whole body (DMAs included) in a hardware loop that
    executes it `reps` times -- used only for wall-clock timing calibration.
    """
    import contextlib

    nc = bass.Bass()
    x_d = nc.dram_tensor("x2", [128, X2COLS], BF16, kind="ExternalInput")
    w_d = nc.dram_tensor("wf", [128, WCOLS], BF16, kind="ExternalInput")
    o_d = nc.dram_tensor("out", [128, OCOLS], BF16, kind="ExternalOutput")

    with tile.TileContext(nc) as tc:
        with (
            tc.tile_pool(name="persist", bufs=1) as persist,
            tc.tile_pool(name="wpool", bufs=3) as wpool,
            tc.tile_pool(name="btmp", bufs=2) as btmp,
            tc.tile_pool(name="psum", bufs=2, space=bass.MemorySpace.PSUM) as psum,
        ):
            x2 = persist.tile([128, X2COLS], BF16, name="x2t")
            x2a = x2[:]
            ost = persist.tile([128, OCOLS], BF16, name="ostage")
            osa = ost[:]

            loop = (
                tc.For_i(0, reps, 1, hint_engines=(mybir.EngineType.PE,))
                if reps is not None else contextlib.nullcontext()
            )
            with loop:
                _emit_body(nc, x2a, osa, x_d, w_d, o_d, wpool, btmp, psum)
    _split_matmul_waits(nc)
    return nc


def _emit_body(nc, x2a, osa, x_d, w_d, o_d, wpool, btmp, psum):
    nc.sync.dma_start(x2a, x_d[:])

    nch = len(WCHUNKS) - 1
    wc = [None] * nch
    banks = [None] * 16  # psum bank APs (pool cycles 2 physical buffers)
    c2ch = np.searchsorted(WCHUNKS, np.arange(NC_), side="right") - 1

    for c in range(NC_):
        jch = int(c2ch[c])
        c_loc = c - WCHUNKS[jch]
        q, cg = divmod(c, 4)
        g, j = divmod(q, 8)   # bank / pair-slot of position c's A half

        if c_loc == 0:
            ncols = (WCHUNKS[jch + 1] - WCHUNKS[jch]) * 64
            wt = wpool.tile([128, WCMAX * 64], BF16, tag="wc", name=f"wc{jch}")
            wc[jch] = wt[:]
            nc.sync.dma_start(
                _ap(wc[jch], 0, [[WCMAX * 64, 128], [1, ncols]]),
                _ap(w_d[:], WCHUNKS[jch] * 64, [[WCOLS, 128], [1, ncols]]),
            )
        if c % 32 == 0 and c < LP:
            pgt = psum.tile([128, 512], F32, tag="ps", name=f"ps{g}")
            banks[g] = pgt[:]

        lhs = _ap(x2a, c * B, [[X2COLS, 128], [1, B]])
        wofs = c_loc * 64
        fused = 4 <= c < LP and j != 0
        if fused:
            # one N=64 matmul: [B half of pos c-4 | A half of pos c]
            nc.tensor.matmul(
                _ap(banks[g], 32 * cg * 512 + 64 * j - 32, [[512, 32], [1, 64]]),
                lhs,
                _ap(wc[jch], wofs, [[WCMAX * 64, 128], [1, 64]]),
                start=True, stop=True,
                tile_position=(0, 32 * cg), skip_group_check=True,
            )
        else:
            if c >= 4:
                # B half of pos c-4 -> pair-slot 7 of the previous bank
                nc.tensor.matmul(
                    _ap(banks[(c - 4) // 32], 32 * cg * 512 + 480,
                        [[512, 32], [1, 32]]),
                    lhs,
                    _ap(wc[jch], wofs, [[WCMAX * 64, 128], [1, 32]]),
                    start=True, stop=True,
                    tile_position=(0, 32 * cg), skip_group_check=True,
                )
            if c < LP:
                # A half of pos c -> pair-slot 0 of the current bank
                nc.tensor.matmul(
                    _ap(banks[g], 32 * cg * 512 + 64 * j, [[512, 32], [1, 32]]),
                    lhs,
                    _ap(wc[jch], wofs + 32, [[WCMAX * 64, 128], [1, 32]]),
                    start=True, stop=True,
                    tile_position=(0, 32 * cg), skip_group_check=True,
                )

        if c >= 35 and c % 32 == 3:
            gd = (c - 35) // 32  # bank whose last write (B of pos 32gd+31) was c
            # TensorTensor may read only ONE input from PSUM: stage the B
            # halves into SBUF first (bf16), then add the A halves (PSUM).
            bt = btmp.tile([128, 256], BF16, tag="bt", name=f"bt{gd}")
            bta = bt[:]
            nc.vector.tensor_copy(
                _ap(bta, 0, [[256, 128], [32, 8], [1, 32]]),
                _ap(banks[gd], 32, [[512, 128], [64, 8], [1, 32]]),
            )
            nc.vector.tensor_tensor(
                _ap(osa, gd * 256, [[OCOLS, 128], [32, 8], [1, 32]]),
                _ap(banks[gd], 0, [[512, 128], [64, 8], [1, 32]]),
                _ap(bta, 0, [[256, 128], [32, 8], [1, 32]]),
                mybir.AluOpType.add,
            )
            if gd + 1 in OGROUPS:
                k = OGROUPS.index(gd + 1) - 1
                lo, hi = OGROUPS[k] * 256, OGROUPS[k + 1] * 256
                # ACT's HWDGE ring: a waiting output DMA must not block the
                # weight-chunk DMAs queued behind it on SP's FIFO ring.
                nc.scalar.dma_start(
                    _ap(o_d[:], lo, [[OCOLS, 128], [1, hi - lo]]),
                    _ap(osa, lo, [[OCOLS, 128], [1, hi - lo]]),
                )


def _split_matmul_waits(nc):
    """This walrus build allows at most one sync wait per instruction.
    Relocate each multi-wait instruction's waits onto a chain of single-wait
    NoOps inserted just before it on the same engine -- program order makes
    this semantically identical."""
    for f in nc.m.functions:
        for bb in f.blocks:
            insts = list(bb.instructions)
            out = []
            changed = False
            for ins in insts:
                si = ins.sync_info
                if (si is not None and si.on_wait
                        and len(si.on_wait) >= 2):
                    for w in si.on_wait:
                        nop = mybir.InstNoOp(
                            name=nc.get_next_instruction_name(),
                            ins=[], outs=[],
                            sync_info=mybir.SyncInfo(
                                on_wait=[w], on_update=[]),
                            bass_nofuse=True,
                            engine=ins.engine,
                        )
                        nc.inst_map[nop.name] = nop
                        out.append(nop)
                    ins.sync_info = mybir.SyncInfo(
                        on_wait=[], on_update=list(si.on_update))
                    changed = True
                out.append(ins)
            if changed:
                bb.instructions = out


def _get_nc():
    if "nc" not in _CACHE:
        _CACHE["nc"] = _emit()
    return _CACHE["nc"]


def _shard_inputs(x, weight):
    """Pre-permute full inputs into the per-core bf16 kernel layouts."""
    import ml_dtypes

    bf16 = ml_dtypes.bfloat16
    x = np.asarray(x, dtype=np.float32)
    weight = np.asarray(weight, dtype=np.float32)

    xpad = np.zeros((B, IC, NCORES * LP + NC_ + 4), dtype=np.float32)
    xpad[:, :, :L] = x
    # wglob: 4-zero L prefix so the B-half window (l0+c-4) never underflows,
    # an 8th zero tap, and an L suffix past L_OUT.
    wglob = np.zeros((OC, IC, 4 + NCORES * LP + NC_, 8), dtype=np.float32)
    wglob[:, :, 4 : 4 + L_OUT, :K] = weight

    in_maps = []
    for m in range(NCORES):
        l0 = m * LP
        x2 = np.empty((4, IC, NC_, B), dtype=bf16)
        for kk in range(4):
            x2[kk] = xpad[:, :, l0 + kk : l0 + kk + NC_].transpose(1, 2, 0)
        wA = wglob[:, :, 4 + l0 : 4 + l0 + NC_, 0:4]  # (O,I,c,kk)
        wB = wglob[:, :, l0 : l0 + NC_, 4:8]
        wf = np.empty((4, IC, NC_, 2, OC), dtype=bf16)
        wf[:, :, :, 0, :] = wB.transpose(3, 1, 2, 0)
        wf[:, :, :, 1, :] = wA.transpose(3, 1, 2, 0)
        in_maps.append({
            "x2": np.ascontiguousarray(x2).reshape(128, X2COLS),
            "wf": np.ascontiguousarray(wf).reshape(128, WCOLS),
        })
    return in_maps


def _unshard_output(res):
    """res: list of per-core {"out": (128, OCOLS) bf16} -> (B, OC, L_OUT) f32."""
    out = np.empty((B, OC, NCORES * LP), dtype=np.float32)
    for m in range(NCORES):
        arr = np.asarray(res[m]["out"], dtype=np.float32)
        arr = arr.reshape(4, B, 16, 8, OC)          # (cg, b, g, j, o)
        out[:, :, m * LP : (m + 1) * LP] = (
            arr.transpose(1, 4, 2, 3, 0).reshape(B, OC, LP)
        )
    return np.ascontiguousarray(out[:, :, :L_OUT])


def kernel(x, weight):
    nc = _get_nc()
    in_maps = _shard_inputs(x, weight)
    res = run_bass_kernel_spmd(nc, in_maps, list(range(NCORES))).results
    return _unshard_output(res)


# revision 21
# speedup vs baseline: 5.1795x; 2.2364x over previous
"""LocallyConnected1d (B=32, C=32, L=4096, K=7, stride=1) Trainium2 Bass kernel.

Strategy (hardcoded for this problem):
  - Shard L_out=4090 across 8 cores (sequence parallel), 512 positions/core
    (padded; core 7 carries 6 zero-padded positions). Weights are read from
    HBM exactly once, in bf16 (rel-err budget 2e-2 >> bf16 rounding).
  - Host pre-permutes operands into PE-friendly bf16 layouts:
      x2 [128, 516*32]: partition (tap-band kk 0..3, in_C i), col (c, b),
                        value x[b, i, 512m + c + kk]
      weights, col (c, o):  A half  w[o, i, 512m+c, kk]     (128 partitions)
                            B half  w[o, i, 512m+c-4, 4+kk] (96 partitions)
  - One stationary column c (one LDWEIGHTS of x2[:, c*32:+32]) serves TWO
    positions: taps 0..3 of position c (A) and taps 4..6 of position c-4 (B).
  - WSPLIT=True: A and B live in separate dram tensors (wa 128p / wb 96p) so
    no zero tap-band is ever transferred; two N=32 matmuls per column.
    WSPLIT=False: single interleaved tensor wf [128, 516*64] with a zeroed
    4th B tap band; one fused N=64 matmul per column where slots allow.
  - PSUM: position p -> col group cg=p%4 (psum partitions 32cg..32cg+31
    hold b), bank g=p//32, pair-slot j=(p//4)%8: A half at cols
    [64j,64j+32), B half at [64j+32,64j+64). Every psum element is written
    by exactly ONE matmul (start=stop=True) -- no accumulation groups, no
    has_written hazards. VectorE stages B to SBUF (bf16) and adds A (PSUM)
    into the bf16 output staging tile; the host just un-permutes + upcasts.
  - Output leaves in 4 chunked bf16 DMAs on ACT's HWDGE ring (a waiting
    output DMA must not block weight DMAs queued on SP's FIFO ring).
"""

import sys

if "/opt/trn_rl_repo" not in sys.path:
    sys.path.insert(0, "/opt/trn_rl_repo")

import numpy as np

import bass_rust
from concourse import bass, mybir, tile
from concourse.bass_utils import run_bass_kernel_spmd

# Problem constants (hardcoded; must match the grading reference).
B = 32          # batch
IC = 32         # in channels
L = 4096        # input length
OC = 32         # out channels
K = 7           # kernel taps
L_OUT = 4090    # (L - (K-1)) // 1

NCORES = 8
LP = 512        # positions per core (padded: 8*512 = 4096 >= 4090)
NC_ = LP + 4    # stationary columns per core (c = 0..515)
WSPLIT = False  # split A/B weight tensors (no zero tap band transferred)
DBUF = True     # double-buffer x2/ostage across timing-loop iterations
# weight chunk column boundaries: big steady-state chunks, small final chunk
# so the tail (matmuls that can only start after the last chunk lands) is short
WCHUNKS = [0, 120, 240, 340, 420, 470, 500, 516]
WCMAX = 120     # max chunk width (tile allocation size)
# output DMA bank-group boundaries (last group small -> small tail DMA)
OGROUPS = [0, 5, 10, 14, 16]
PSUM_BUFS = 6
BTMP_BUFS = 4

X2COLS = NC_ * B        # x2 cols: c*32 + b
WCOLS = NC_ * 64        # wf cols (fused layout): c*64 + h*32 + o
WSCOLS = NC_ * 32       # wa/wb cols (split layout): c*32 + o
OCOLS = 16 * 256        # ostage cols: g*256 + j*32 + o   (= 4096)

F32 = mybir.dt.float32
BF16 = mybir.dt.bfloat16

_CACHE = {}


def _ap(t_ap, offset, dims):
    """Build a raw access pattern on the tensor behind an AP."""
    return bass_rust.AP(t_ap.tensor, int(offset), [[int(s), int(n)] for s, n in dims])


def _emit(reps=None, mode="full"):
    """Build the (identical-per-core) single-core program.

    reps: if set, wrap the whole body (DMAs included) in a hardware loop that
    executes it `reps` times -- used only for wall-clock timing calibration.
    mode: "full" | "pe" (no DMAs: PE/DVE only) | "dma" (DMAs only) --
    hardware profiling probes.
    """
    import contextlib

    nc = bass.Bass()
    x_d = nc.dram_tensor("x2", [128, X2COLS], BF16, kind="ExternalInput")
    if WSPLIT:
        w_d = (nc.dram_tensor("wa", [128, WSCOLS], BF16, kind="ExternalInput"),
               nc.dram_tensor("wb", [96, WSCOLS], BF16, kind="ExternalInput"))
    else:
        w_d = nc.dram_tensor("wf", [128, WCOLS], BF16, kind="ExternalInput")
    o_d = nc.dram_tensor("out", [128, OCOLS], BF16, kind="ExternalOutput")

    with tile.TileContext(nc) as tc:
        with (
            tc.tile_pool(name="x2pool", bufs=2 if DBUF else 1) as x2pool,
            tc.tile_pool(name="wpool", bufs=4) as wpool,
            tc.tile_pool(name="btmp", bufs=BTMP_BUFS) as btmp,
            tc.tile_pool(name="psum", bufs=PSUM_BUFS,
                         space=bass.MemorySpace.PSUM) as psum,
        ):
            if not DBUF:
                x2 = x2pool.tile([128, X2COLS], BF16, name="x2t")
                ost = x2pool.tile([128, OCOLS], BF16, name="ostage")
            loop = (
                tc.For_i(0, reps, 1, hint_engines=(mybir.EngineType.PE,))
                if reps is not None else contextlib.nullcontext()
            )
            with loop:
                if DBUF:
                    x2 = x2pool.tile([128, X2COLS], BF16, tag="x2t", name="x2t")
                    ost = x2pool.tile([128, OCOLS], BF16, tag="ost", name="ostage")
                _emit_body(nc, x2[:], ost[:], x_d, w_d, o_d, wpool, btmp,
                           psum, mode)
    _split_matmul_waits(nc)
    return nc


def _emit_body(nc, x2a, osa, x_d, w_d, o_d, wpool, btmp, psum, mode="full"):
    dmas = mode in ("full", "dma")
    compute = mode in ("full", "pe")
    if dmas:
        nc.sync.dma_start(x2a, x_d[:])
    else:
        # probe mode: 1-col DMA just to mark the tile written/allocated
        nc.sync.dma_start(_ap(x2a, 0, [[X2COLS, 128], [1, 32]]),
                          _ap(x_d[:], 0, [[X2COLS, 128], [1, 32]]))
    if not compute:
        nc.vector.memset(_ap(osa, 0, [[OCOLS, 128], [1, 32]]), 0)

    nch = len(WCHUNKS) - 1
    wca = [None] * nch
    wcb = [None] * nch
    banks = [None] * 16  # psum bank APs (pool cycles 6 physical buffers)
    c2ch = np.searchsorted(WCHUNKS, np.arange(NC_), side="right") - 1
    wcw = WCMAX * (32 if WSPLIT else 64)   # chunk tile cols (A part)

    for c in range(NC_):
        jch = int(c2ch[c])
        c_loc = c - WCHUNKS[jch]
        q, cg = divmod(c, 4)
        g, j = divmod(q, 8)   # bank / pair-slot of position c's A half

        if c_loc == 0:
            nc_cols = WCHUNKS[jch + 1] - WCHUNKS[jch]
            if WSPLIT:
                wta = wpool.tile([128, wcw], BF16, tag="wca", name=f"wca{jch}")
                wtb = wpool.tile([128, wcw], BF16, tag="wcb", name=f"wcb{jch}")
                wca[jch], wcb[jch] = wta[:], wtb[:]
                ncols = nc_cols * 32 if dmas else 32
                nc.sync.dma_start(
                    _ap(wca[jch], 0, [[wcw, 128], [1, ncols]]),
                    _ap(w_d[0][:], WCHUNKS[jch] * 32, [[WSCOLS, 128], [1, ncols]]),
                )
                nc.sync.dma_start(
                    _ap(wcb[jch], 0, [[wcw, 96], [1, ncols]]),
                    _ap(w_d[1][:], WCHUNKS[jch] * 32, [[WSCOLS, 96], [1, ncols]]),
                )
            else:
                wt = wpool.tile([128, wcw], BF16, tag="wca", name=f"wca{jch}")
                wca[jch] = wt[:]
                ncols = nc_cols * 64 if dmas else 32
                nc.sync.dma_start(
                    _ap(wca[jch], 0, [[wcw, 128], [1, ncols]]),
                    _ap(w_d[:], WCHUNKS[jch] * 64, [[WCOLS, 128], [1, ncols]]),
                )
        if not compute:
            if c >= 35 and c % 32 == 3 and (c - 35) // 32 + 1 in OGROUPS:
                gd = (c - 35) // 32
                k = OGROUPS.index(gd + 1) - 1
                lo, hi = OGROUPS[k] * 256, OGROUPS[k + 1] * 256
                nc.scalar.dma_start(
                    _ap(o_d[:], lo, [[OCOLS, 128], [1, hi - lo]]),
                    _ap(osa, lo, [[OCOLS, 128], [1, hi - lo]]),
                )
            continue
        if c % 32 == 0 and c < LP:
            pgt = psum.tile([128, 512], F32, tag="ps", name=f"ps{g}")
            banks[g] = pgt[:]

        lhs = _ap(x2a, c * B, [[X2COLS, 128], [1, B]])
        if WSPLIT:
            if c >= 4:
                # B half of pos c-4 -> pair-slot 7 of prev bank / j-1 of cur
                gb, jb = ((c - 4) // 32, 7) if j == 0 else (g, j - 1)
                nc.tensor.matmul(
                    _ap(banks[gb], 32 * cg * 512 + 64 * jb + 32,
                        [[512, 32], [1, 32]]),
                    _ap(x2a, c * B, [[X2COLS, 96], [1, B]]),
                    _ap(wcb[jch], c_loc * 32, [[wcw, 96], [1, 32]]),
                    start=True, stop=True,
                    tile_position=(0, 32 * cg), skip_group_check=True,
                )
            if c < LP:
                nc.tensor.matmul(
                    _ap(banks[g], 32 * cg * 512 + 64 * j, [[512, 32], [1, 32]]),
                    lhs,
                    _ap(wca[jch], c_loc * 32, [[wcw, 128], [1, 32]]),
                    start=True, stop=True,
                    tile_position=(0, 32 * cg), skip_group_check=True,
                )
        else:
            wofs = c_loc * 64
            fused = 4 <= c < LP and j != 0
            if fused:
                # one N=64 matmul: [B half of pos c-4 | A half of pos c]
                nc.tensor.matmul(
                    _ap(banks[g], 32 * cg * 512 + 64 * j - 32,
                        [[512, 32], [1, 64]]),
                    lhs,
                    _ap(wca[jch], wofs, [[wcw, 128], [1, 64]]),
                    start=True, stop=True,
                    tile_position=(0, 32 * cg), skip_group_check=True,
                )
            else:
                if c >= 4:
                    nc.tensor.matmul(
                        _ap(banks[(c - 4) // 32], 32 * cg * 512 + 480,
                            [[512, 32], [1, 32]]),
                        lhs,
                        _ap(wca[jch], wofs, [[wcw, 128], [1, 32]]),
                        start=True, stop=True,
                        tile_position=(0, 32 * cg), skip_group_check=True,
                    )
                if c < LP:
                    nc.tensor.matmul(
                        _ap(banks[g], 32 * cg * 512 + 64 * j,
                            [[512, 32], [1, 32]]),
                        lhs,
                        _ap(wca[jch], wofs + 32, [[wcw, 128], [1, 32]]),
                        start=True, stop=True,
                        tile_position=(0, 32 * cg), skip_group_check=True,
                    )

        if c >= 35 and c % 32 == 3:
            gd = (c - 35) // 32  # bank whose last write (B of pos 32gd+31) was c
            # TensorTensor may read only ONE input from PSUM: stage the B
            # halves into SBUF first (bf16), then add the A halves (PSUM).
            bt = btmp.tile([128, 256], BF16, tag="bt", name=f"bt{gd}")
            bta = bt[:]
            nc.vector.tensor_copy(
                _ap(bta, 0, [[256, 128], [32, 8], [1, 32]]),
                _ap(banks[gd], 32, [[512, 128], [64, 8], [1, 32]]),
            )
            nc.vector.tensor_tensor(
                _ap(osa, gd * 256, [[OCOLS, 128], [32, 8], [1, 32]]),
                _ap(banks[gd], 0, [[512, 128], [64, 8], [1, 32]]),
                _ap(bta, 0, [[256, 128], [32, 8], [1, 32]]),
                mybir.AluOpType.add,
            )
            if dmas and gd + 1 in OGROUPS:
                k = OGROUPS.index(gd + 1) - 1
                lo, hi = OGROUPS[k] * 256, OGROUPS[k + 1] * 256
                # ACT's HWDGE ring: a waiting output DMA must not block the
                # weight-chunk DMAs queued behind it on SP's FIFO ring.
                nc.scalar.dma_start(
                    _ap(o_d[:], lo, [[OCOLS, 128], [1, hi - lo]]),
                    _ap(osa, lo, [[OCOLS, 128], [1, hi - lo]]),
                )


def _split_matmul_waits(nc):
    """This walrus build allows at most one sync wait per instruction.
    Relocate each multi-wait instruction's waits onto a chain of single-wait
    NoOps inserted just before it on the same engine -- program order makes
    this semantically identical."""
    for f in nc.m.functions:
        for bb in f.blocks:
            insts = list(bb.instructions)
            out = []
            changed = False
            for ins in insts:
                si = ins.sync_info
                if (si is not None and si.on_wait
                        and len(si.on_wait) >= 2):
                    for w in si.on_wait:
                        nop = mybir.InstNoOp(
                            name=nc.get_next_instruction_name(),
                            ins=[], outs=[],
                            sync_info=mybir.SyncInfo(
                                on_wait=[w], on_update=[]),
                            bass_nofuse=True,
                            engine=ins.engine,
                        )
                        nc.inst_map[nop.name] = nop
                        out.append(nop)
                    ins.sync_info = mybir.SyncInfo(
                        on_wait=[], on_update=list(si.on_update))
                    changed = True
                out.append(ins)
            if changed:
                bb.instructions = out


def _get_nc():
    key = ("nc", WSPLIT)
    if key not in _CACHE:
        _CACHE[key] = _emit()
    return _CACHE[key]


def _shard_inputs(x, weight):
    """Pre-permute full inputs into the per-core bf16 kernel layouts."""
    import ml_dtypes

    bf16 = ml_dtypes.bfloat16
    x = np.asarray(x, dtype=np.float32)
    weight = np.asarray(weight, dtype=np.float32)

    xpad = np.zeros((B, IC, NCORES * LP + NC_ + 4), dtype=np.float32)
    xpad[:, :, :L] = x
    # wglob: 4-zero L prefix so the B-half window (l0+c-4) never underflows,
    # an 8th zero tap, and an L suffix past L_OUT.
    wglob = np.zeros((OC, IC, 4 + NCORES * LP + NC_, 8), dtype=np.float32)
    wglob[:, :, 4 : 4 + L_OUT, :K] = weight

    in_maps = []
    for m in range(NCORES):
        l0 = m * LP
        x2 = np.empty((4, IC, NC_, B), dtype=bf16)
        for kk in range(4):
            x2[kk] = xpad[:, :, l0 + kk : l0 + kk + NC_].transpose(1, 2, 0)
        wA = wglob[:, :, 4 + l0 : 4 + l0 + NC_, 0:4]  # (O,I,c,kk)
        wB = wglob[:, :, l0 : l0 + NC_, 4:8]
        im = {"x2": np.ascontiguousarray(x2).reshape(128, X2COLS)}
        if WSPLIT:
            im["wa"] = np.ascontiguousarray(
                wA.transpose(3, 1, 2, 0).astype(bf16)).reshape(128, WSCOLS)
            im["wb"] = np.ascontiguousarray(
                wB[:, :, :, :3].transpose(3, 1, 2, 0).astype(bf16)
            ).reshape(96, WSCOLS)
        else:
            wf = np.empty((4, IC, NC_, 2, OC), dtype=bf16)
            wf[:, :, :, 0, :] = wB.transpose(3, 1, 2, 0)
            wf[:, :, :, 1, :] = wA.transpose(3, 1, 2, 0)
            im["wf"] = np.ascontiguousarray(wf).reshape(128, WCOLS)
        in_maps.append(im)
    return in_maps


def _unshard_output(res):
    """res: list of per-core {"out": (128, OCOLS) bf16} -> (B, OC, L_OUT) f32."""
    out = np.empty((B, OC, NCORES * LP), dtype=np.float32)
    for m in range(NCORES):
        arr = np.asarray(res[m]["out"], dtype=np.float32)
        arr = arr.reshape(4, B, 16, 8, OC)          # (cg, b, g, j, o)
        out[:, :, m * LP : (m + 1) * LP] = (
            arr.transpose(1, 4, 2, 3, 0).reshape(B, OC, LP)
        )
    return np.ascontiguousarray(out[:, :, :L_OUT])


def kernel(x, weight):
    nc = _get_nc()
    in_maps = _shard_inputs(x, weight)
    res = run_bass_kernel_spmd(nc, in_maps, list(range(NCORES))).results
    return _unshard_output(res)


# revision 23
# speedup vs baseline: 5.3633x; 1.0355x over previous
"""LocallyConnected1d (B=32, C=32, L=4096, K=7, stride=1) Trainium2 Bass kernel.

Strategy (hardcoded for this problem):
  - Shard L_out=4090 across 8 cores (sequence parallel), 512 positions/core
    (padded; core 7 carries 6 zero-padded positions). Weights are read from
    HBM exactly once, in bf16 (rel-err budget 2e-2 >> bf16 rounding).
  - Host pre-permutes operands into PE-friendly bf16 layouts:
      x2 [128, 516*32]: partition (tap-band kk 0..3, in_C i), col (c, b),
                        value x[b, i, 512m + c + kk]
      weights, col (c, o):  A half  w[o, i, 512m+c, kk]     (128 partitions)
                            B half  w[o, i, 512m+c-4, 4+kk] (96 partitions)
  - One stationary column c (one LDWEIGHTS of x2[:, c*32:+32]) serves TWO
    positions: taps 0..3 of position c (A) and taps 4..6 of position c-4 (B).
  - WSPLIT=True: A and B live in separate dram tensors (wa 128p / wb 96p) so
    no zero tap-band is ever transferred; two N=32 matmuls per column.
    WSPLIT=False: single interleaved tensor wf [128, 516*64] with a zeroed
    4th B tap band; one fused N=64 matmul per column where slots allow.
  - PSUM: position p -> col group cg=p%4 (psum partitions 32cg..32cg+31
    hold b), bank g=p//32, pair-slot j=(p//4)%8: A half at cols
    [64j,64j+32), B half at [64j+32,64j+64). Every psum element is written
    by exactly ONE matmul (start=stop=True) -- no accumulation groups, no
    has_written hazards. VectorE stages B to SBUF (bf16) and adds A (PSUM)
    into the bf16 output staging tile; the host just un-permutes + upcasts.
  - Output leaves in 4 chunked bf16 DMAs on ACT's HWDGE ring (a waiting
    output DMA must not block weight DMAs queued on SP's FIFO ring).
"""

import sys

if "/opt/trn_rl_repo" not in sys.path:
    sys.path.insert(0, "/opt/trn_rl_repo")

import numpy as np

import bass_rust
from concourse import bass, mybir, tile
from concourse.bass_utils import run_bass_kernel_spmd

# Problem constants (hardcoded; must match the grading reference).
B = 32          # batch
IC = 32         # in channels
L = 4096        # input length
OC = 32         # out channels
K = 7           # kernel taps
L_OUT = 4090    # (L - (K-1)) // 1

NCORES = 8
LP = 512        # positions per core (padded: 8*512 = 4096 >= 4090)
NC_ = LP + 4    # stationary columns per core (c = 0..515)
WSPLIT = False  # split A/B weight tensors (no zero tap band transferred)
DBUF = True     # double-buffer x2/ostage across timing-loop iterations
# weight chunk column boundaries: big steady-state chunks, small final chunk
# so the tail (matmuls that can only start after the last chunk lands) is short
WCHUNKS = [0, 120, 240, 340, 420, 470, 500, 516]
WCMAX = 120     # max chunk width (tile allocation size)
# output DMA bank-group boundaries (last group small -> small tail DMA)
OGROUPS = [0, 5, 10, 14, 16]
PSUM_BUFS = 6
BTMP_BUFS = 4
WPOOL_BUFS = 6

X2COLS = NC_ * B        # x2 cols: c*32 + b
WCOLS = NC_ * 64        # wf cols (fused layout): c*64 + h*32 + o
WSCOLS = NC_ * 32       # wa/wb cols (split layout): c*32 + o
OCOLS = 16 * 256        # ostage cols: g*256 + j*32 + o   (= 4096)

F32 = mybir.dt.float32
BF16 = mybir.dt.bfloat16

_CACHE = {}


def _ap(t_ap, offset, dims):
    """Build a raw access pattern on the tensor behind an AP."""
    return bass_rust.AP(t_ap.tensor, int(offset), [[int(s), int(n)] for s, n in dims])


def _emit(reps=None, mode="full"):
    """Build the (identical-per-core) single-core program.

    reps: if set, wrap the whole body (DMAs included) in a hardware loop that
    executes it `reps` times -- used only for wall-clock timing calibration.
    mode: "full" | "pe" (no DMAs: PE/DVE only) | "dma" (DMAs only) --
    hardware profiling probes.
    """
    import contextlib

    nc = bass.Bass()
    x_d = nc.dram_tensor("x2", [128, X2COLS], BF16, kind="ExternalInput")
    if WSPLIT:
        w_d = (nc.dram_tensor("wa", [128, WSCOLS], BF16, kind="ExternalInput"),
               nc.dram_tensor("wb", [96, WSCOLS], BF16, kind="ExternalInput"))
    else:
        w_d = nc.dram_tensor("wf", [128, WCOLS], BF16, kind="ExternalInput")
    o_d = nc.dram_tensor("out", [128, OCOLS], BF16, kind="ExternalOutput")

    with tile.TileContext(nc) as tc:
        with (
            tc.tile_pool(name="x2pool", bufs=2 if DBUF else 1) as x2pool,
            tc.tile_pool(name="wpool", bufs=WPOOL_BUFS) as wpool,
            tc.tile_pool(name="btmp", bufs=BTMP_BUFS) as btmp,
            tc.tile_pool(name="psum", bufs=PSUM_BUFS,
                         space=bass.MemorySpace.PSUM) as psum,
        ):
            if not DBUF:
                x2 = x2pool.tile([128, X2COLS], BF16, name="x2t")
                ost = x2pool.tile([128, OCOLS], BF16, name="ostage")
            loop = (
                tc.For_i(0, reps, 1, hint_engines=(mybir.EngineType.PE,))
                if reps is not None else contextlib.nullcontext()
            )
            with loop:
                if DBUF:
                    x2 = x2pool.tile([128, X2COLS], BF16, tag="x2t", name="x2t")
                    ost = x2pool.tile([128, OCOLS], BF16, tag="ost", name="ostage")
                _emit_body(nc, x2[:], ost[:], x_d, w_d, o_d, wpool, btmp,
                           psum, mode)
    _split_matmul_waits(nc)
    return nc


def _emit_body(nc, x2a, osa, x_d, w_d, o_d, wpool, btmp, psum, mode="full"):
    dmas = mode in ("full", "dma")
    compute = mode in ("full", "pe")
    if dmas:
        nc.sync.dma_start(x2a, x_d[:])
    else:
        # probe mode: 1-col DMA just to mark the tile written/allocated
        nc.sync.dma_start(_ap(x2a, 0, [[X2COLS, 128], [1, 32]]),
                          _ap(x_d[:], 0, [[X2COLS, 128], [1, 32]]))
    if not compute:
        nc.vector.memset(_ap(osa, 0, [[OCOLS, 128], [1, 32]]), 0)

    nch = len(WCHUNKS) - 1
    wca = [None] * nch
    wcb = [None] * nch
    banks = [None] * 16  # psum bank APs (pool cycles 6 physical buffers)
    c2ch = np.searchsorted(WCHUNKS, np.arange(NC_), side="right") - 1
    wcw = WCMAX * (32 if WSPLIT else 64)   # chunk tile cols (A part)

    for c in range(NC_):
        jch = int(c2ch[c])
        c_loc = c - WCHUNKS[jch]
        q, cg = divmod(c, 4)
        g, j = divmod(q, 8)   # bank / pair-slot of position c's A half

        if c_loc == 0:
            nc_cols = WCHUNKS[jch + 1] - WCHUNKS[jch]
            if WSPLIT:
                wta = wpool.tile([128, wcw], BF16, tag="wca", name=f"wca{jch}")
                wtb = wpool.tile([128, wcw], BF16, tag="wcb", name=f"wcb{jch}")
                wca[jch], wcb[jch] = wta[:], wtb[:]
                ncols = nc_cols * 32 if dmas else 32
                nc.sync.dma_start(
                    _ap(wca[jch], 0, [[wcw, 128], [1, ncols]]),
                    _ap(w_d[0][:], WCHUNKS[jch] * 32, [[WSCOLS, 128], [1, ncols]]),
                )
                nc.sync.dma_start(
                    _ap(wcb[jch], 0, [[wcw, 96], [1, ncols]]),
                    _ap(w_d[1][:], WCHUNKS[jch] * 32, [[WSCOLS, 96], [1, ncols]]),
                )
            else:
                wt = wpool.tile([128, wcw], BF16, tag="wca", name=f"wca{jch}")
                wca[jch] = wt[:]
                ncols = nc_cols * 64 if dmas else 32
                nc.sync.dma_start(
                    _ap(wca[jch], 0, [[wcw, 128], [1, ncols]]),
                    _ap(w_d[:], WCHUNKS[jch] * 64, [[WCOLS, 128], [1, ncols]]),
                )
        if not compute:
            if c >= 35 and c % 32 == 3 and (c - 35) // 32 + 1 in OGROUPS:
                gd = (c - 35) // 32
                k = OGROUPS.index(gd + 1) - 1
                lo, hi = OGROUPS[k] * 256, OGROUPS[k + 1] * 256
                nc.scalar.dma_start(
                    _ap(o_d[:], lo, [[OCOLS, 128], [1, hi - lo]]),
                    _ap(osa, lo, [[OCOLS, 128], [1, hi - lo]]),
                )
            continue
        if c % 32 == 0 and c < LP:
            pgt = psum.tile([128, 512], F32, tag="ps", name=f"ps{g}")
            banks[g] = pgt[:]

        lhs = _ap(x2a, c * B, [[X2COLS, 128], [1, B]])
        if WSPLIT:
            if c >= 4:
                # B half of pos c-4 -> pair-slot 7 of prev bank / j-1 of cur
                gb, jb = ((c - 4) // 32, 7) if j == 0 else (g, j - 1)
                nc.tensor.matmul(
                    _ap(banks[gb], 32 * cg * 512 + 64 * jb + 32,
                        [[512, 32], [1, 32]]),
                    _ap(x2a, c * B, [[X2COLS, 96], [1, B]]),
                    _ap(wcb[jch], c_loc * 32, [[wcw, 96], [1, 32]]),
                    start=True, stop=True,
                    tile_position=(0, 32 * cg), skip_group_check=True,
                )
            if c < LP:
                nc.tensor.matmul(
                    _ap(banks[g], 32 * cg * 512 + 64 * j, [[512, 32], [1, 32]]),
                    lhs,
                    _ap(wca[jch], c_loc * 32, [[wcw, 128], [1, 32]]),
                    start=True, stop=True,
                    tile_position=(0, 32 * cg), skip_group_check=True,
                )
        else:
            wofs = c_loc * 64
            fused = 4 <= c < LP and j != 0
            if fused:
                # one N=64 matmul: [B half of pos c-4 | A half of pos c]
                nc.tensor.matmul(
                    _ap(banks[g], 32 * cg * 512 + 64 * j - 32,
                        [[512, 32], [1, 64]]),
                    lhs,
                    _ap(wca[jch], wofs, [[wcw, 128], [1, 64]]),
                    start=True, stop=True,
                    tile_position=(0, 32 * cg), skip_group_check=True,
                )
            else:
                if c >= 4:
                    nc.tensor.matmul(
                        _ap(banks[(c - 4) // 32], 32 * cg * 512 + 480,
                            [[512, 32], [1, 32]]),
                        lhs,
                        _ap(wca[jch], wofs, [[wcw, 128], [1, 32]]),
                        start=True, stop=True,
                        tile_position=(0, 32 * cg), skip_group_check=True,
                    )
                if c < LP:
                    nc.tensor.matmul(
                        _ap(banks[g], 32 * cg * 512 + 64 * j,
                            [[512, 32], [1, 32]]),
                        lhs,
                        _ap(wca[jch], wofs + 32, [[wcw, 128], [1, 32]]),
                        start=True, stop=True,
                        tile_position=(0, 32 * cg), skip_group_check=True,
                    )

        if c >= 35 and c % 32 == 3:
            gd = (c - 35) // 32  # bank whose last write (B of pos 32gd+31) was c
            # TensorTensor may read only ONE input from PSUM: stage the B
            # halves into SBUF first (bf16), then add the A halves (PSUM).
            bt = btmp.tile([128, 256], BF16, tag="bt", name=f"bt{gd}")
            bta = bt[:]
            nc.vector.tensor_copy(
                _ap(bta, 0, [[256, 128], [32, 8], [1, 32]]),
                _ap(banks[gd], 32, [[512, 128], [64, 8], [1, 32]]),
            )
            nc.vector.tensor_tensor(
                _ap(osa, gd * 256, [[OCOLS, 128], [32, 8], [1, 32]]),
                _ap(banks[gd], 0, [[512, 128], [64, 8], [1, 32]]),
                _ap(bta, 0, [[256, 128], [32, 8], [1, 32]]),
                mybir.AluOpType.add,
            )
            if dmas and gd + 1 in OGROUPS:
                k = OGROUPS.index(gd + 1) - 1
                lo, hi = OGROUPS[k] * 256, OGROUPS[k + 1] * 256
                # ACT's HWDGE ring: a waiting output DMA must not block the
                # weight-chunk DMAs queued behind it on SP's FIFO ring.
                nc.scalar.dma_start(
                    _ap(o_d[:], lo, [[OCOLS, 128], [1, hi - lo]]),
                    _ap(osa, lo, [[OCOLS, 128], [1, hi - lo]]),
                )


def _split_matmul_waits(nc):
    """This walrus build allows at most one sync wait per instruction.
    Relocate each multi-wait instruction's waits onto a chain of single-wait
    NoOps inserted just before it on the same engine -- program order makes
    this semantically identical."""
    for f in nc.m.functions:
        for bb in f.blocks:
            insts = list(bb.instructions)
            out = []
            changed = False
            for ins in insts:
                si = ins.sync_info
                if (si is not None and si.on_wait
                        and len(si.on_wait) >= 2):
                    for w in si.on_wait:
                        nop = mybir.InstNoOp(
                            name=nc.get_next_instruction_name(),
                            ins=[], outs=[],
                            sync_info=mybir.SyncInfo(
                                on_wait=[w], on_update=[]),
                            bass_nofuse=True,
                            engine=ins.engine,
                        )
                        nc.inst_map[nop.name] = nop
                        out.append(nop)
                    ins.sync_info = mybir.SyncInfo(
                        on_wait=[], on_update=list(si.on_update))
                    changed = True
                out.append(ins)
            if changed:
                bb.instructions = out


def _get_nc():
    key = ("nc", WSPLIT)
    if key not in _CACHE:
        _CACHE[key] = _emit()
    return _CACHE[key]


def _shard_inputs(x, weight):
    """Pre-permute full inputs into the per-core bf16 kernel layouts."""
    import ml_dtypes

    bf16 = ml_dtypes.bfloat16
    x = np.asarray(x, dtype=np.float32)
    weight = np.asarray(weight, dtype=np.float32)

    xpad = np.zeros((B, IC, NCORES * LP + NC_ + 4), dtype=np.float32)
    xpad[:, :, :L] = x
    # wglob: 4-zero L prefix so the B-half window (l0+c-4) never underflows,
    # an 8th zero tap, and an L suffix past L_OUT.
    wglob = np.zeros((OC, IC, 4 + NCORES * LP + NC_, 8), dtype=np.float32)
    wglob[:, :, 4 : 4 + L_OUT, :K] = weight

    in_maps = []
    for m in range(NCORES):
        l0 = m * LP
        x2 = np.empty((4, IC, NC_, B), dtype=bf16)
        for kk in range(4):
            x2[kk] = xpad[:, :, l0 + kk : l0 + kk + NC_].transpose(1, 2, 0)
        wA = wglob[:, :, 4 + l0 : 4 + l0 + NC_, 0:4]  # (O,I,c,kk)
        wB = wglob[:, :, l0 : l0 + NC_, 4:8]
        im = {"x2": np.ascontiguousarray(x2).reshape(128, X2COLS)}
        if WSPLIT:
            im["wa"] = np.ascontiguousarray(
                wA.transpose(3, 1, 2, 0).astype(bf16)).reshape(128, WSCOLS)
            im["wb"] = np.ascontiguousarray(
                wB[:, :, :, :3].transpose(3, 1, 2, 0).astype(bf16)
            ).reshape(96, WSCOLS)
        else:
            wf = np.empty((4, IC, NC_, 2, OC), dtype=bf16)
            wf[:, :, :, 0, :] = wB.transpose(3, 1, 2, 0)
            wf[:, :, :, 1, :] = wA.transpose(3, 1, 2, 0)
            im["wf"] = np.ascontiguousarray(wf).reshape(128, WCOLS)
        in_maps.append(im)
    return in_maps


def _unshard_output(res):
    """res: list of per-core {"out": (128, OCOLS) bf16} -> (B, OC, L_OUT) f32."""
    out = np.empty((B, OC, NCORES * LP), dtype=np.float32)
    for m in range(NCORES):
        arr = np.asarray(res[m]["out"], dtype=np.float32)
        arr = arr.reshape(4, B, 16, 8, OC)          # (cg, b, g, j, o)
        out[:, :, m * LP : (m + 1) * LP] = (
            arr.transpose(1, 4, 2, 3, 0).reshape(B, OC, LP)
        )
    return np.ascontiguousarray(out[:, :, :L_OUT])


def kernel(x, weight):
    nc = _get_nc()
    in_maps = _shard_inputs(x, weight)
    res = run_bass_kernel_spmd(nc, in_maps, list(range(NCORES))).results
    return _unshard_output(res)
